# revision 14
# baseline (speedup 1.0000x reference)
"""Trainium2 Bass kernel for nn_Classifier_48223892799748 (retrieval_knn).

Computes sim = (D + enc_pm @ cent_pm.T) / 2 where
  enc_pm = sign((samples - 0.5) @ weight.T)  in {+1,-1}
  cent_pm = centroids mapped {0,1} -> {-1,+1}

Sharding: data-parallel over the batch dim (8192 -> 1024 rows per core,
8 cores). weight / centroids replicated.

Device layout: everything is computed transposed ([D, B] / [C, B]) so that
the sign-encoded matmul-1 output tile [128 d, 512 b] feeds matmul-2
directly as the moving operand (contraction over d) with no on-device
transpose.

Matmul-1 runs in fp8e4m3 DoubleRow (2x bf16 MAC rate; the stream floor
is 512 cols @2.4GHz = ~216ns per [256k x 128d x 512b] matmul): samples
are quantized to e4m3 on the host, weights (+/-1) are exact in fp8.
This flips ~85 of the 10000 sign bits per batch row (quantization noise
~0.64 vs proj sigma ~36), moving each output count by at most ~47 of an
allowed ~105 (rel gate 2e-2 at scale 5260): measured rel err 0.0089,
identical on host emulation and hardware. Matmul-2 is exact.

Sign encodings are split across engines so neither gates the PE stream:
  b-chunk 0: ScalarE Sign        -> enc in {+1,-1}; out = 0.5*ps2 + D/2
  b-chunk 1: DVE    is_gt(0)     -> enc in {0,1};   out = ps2 + (D-colsum)/2
(the {0,1} encoding needs the per-centroid colsum correction, shipped as
a tiny [C,1] bias vector computed on host).

Startup choreography (the DMA ramp only delivers ~50->350 GB/s over the
first 7-14us, while a dt-major d-tile burns the full 1MB sample set in
1.7us):
  - the first BLK(=3) d-tiles run J-MAJOR: each j-pass streams one 256KB
    sample chunk against per-j weight slices of all BLK tiles (separate
    `wj` DRAM layout, one 96KB DMA per j), dropping the early demand
    rate ~3x so the real stream can start ~1.5us earlier and run with
    far fewer starvation gaps. Uses all 8 PSUM banks (2*BLK ps1 + 2 ps2).
  - the 12 early transfers (4 wj slices + 8 sample half-chunks) are
    issued in consumption order, strictly alternating between the two
    HWDGE trigger queues (scalar, sync) to balance the ramp.
  - WARM_N dummy N=512 DR matmuls on a memset tile at the head of the PE
    queue: the HAM clock gate only reaches 8/8 (2.4 GHz) after ~3.4us of
    GAPLESS streaming, and the DMA cannot feed real work before ~10us
    (fixed ~7us runtime preamble + ramp).
  - the 1.2MB centroid load is deferred (program-order paced via
    CENT_AT) out of the startup DMA window; MM2 needs it ~150us in.
  - single DMA per 128KB steady-state weight tile (transfers stripe
    across all 16 queues; more triggers only serialize the sync engine).

enc is stored in two tiles split at pair SPLIT_PAIR so matmul-2's first
instructions (which read early pairs) don't dep-serialize on the LAST
d-tile's sign op (the tile dep tracker is coarser than a pair slice).

The MM2 block order is b-major, b=1 first: the earlier chunk's output
path overlaps the later chunk's matmul block. The last block runs as two
256-column halves, and the final half's output is further split into two
128-col quarters processed on ScalarE and DVE in parallel with their
DMAs on different trigger queues, shortening the serial post-matmul tail.

Legacy modes (f32r / bf16_hilo / fp16 / bf16 matmul-1) remain selectable
via MM1_MODE for A/B testing.
"""

import sys

if "/opt/trn_rl_repo" not in sys.path:
    sys.path.insert(0, "/opt/trn_rl_repo")

import ml_dtypes
import numpy as np

import concourse.bass as bass
import concourse.mybir as mybir
import concourse.tile as tile
from concourse import bacc
from concourse.bass_utils import run_bass_kernel_spmd

# The container's `antenv` package is a stub without `axon_hooks`; if tracing
# is ever requested (BASS_TRACE=1), run_bass_kernel_spmd imports it and would
# crash. Provide a stub module (hook=None -> tracing skipped gracefully)
# unless something (e.g. a test harness) registered a real one already.
try:  # pragma: no cover
    import antenv.axon_hooks  # noqa: F401
except ImportError:
    import types as _types

    import antenv as _antenv

    _hooks = _types.ModuleType("antenv.axon_hooks")
    _hook_store = {"h": None}
    _hooks.set_axon_ntff_profile_hook = lambda h: _hook_store.__setitem__("h", h)
    _hooks.get_axon_ntff_profile_hook = lambda: _hook_store["h"]
    sys.modules["antenv.axon_hooks"] = _hooks
    _antenv.axon_hooks = _hooks

BF16 = ml_dtypes.bfloat16
FP8 = ml_dtypes.float8_e4m3

B, IN_F, D, C = 8192, 1024, 10000, 100
N_CORES = 8
B_SH = B // N_CORES          # 1024 batch rows per core
KC = IN_F // 128             # 8 contraction chunks for matmul 1
KCP = KC // 2                # 4 DoubleRow contraction pairs
DT = (D + 127) // 128        # 79 d-tiles
D_PAD = DT * 128             # 10112
NB = B_SH // 512             # 2 psum-width chunks of the local batch
CENTER = 0.5

# matmul-1 mode: "fp8_dr" | "f32r" | "bf16_hilo" | "fp16" | "bf16"
import os as _os
MM1_MODE = _os.environ.get("MM1_MODE", "fp8_dr")
USE_F32R = MM1_MODE == "f32r"
USE_FP8DR = MM1_MODE == "fp8_dr"
# split sign-encoding across ScalarE+DVE (fp8_dr path only)
SIGN_SPLIT = _os.environ.get("SIGN_SPLIT", "1") == "1"
# dummy warm-up matmuls at the head of the PE queue (see module docstring).
# 9 covers the observed 11.0-11.8us arrival spread of the first sample
# chunk: if the dummies run out first, the pre-latch gap resets the HAM
# clock ramp and costs ~2.5us of 1.2GHz matmuls (post-latch gaps are
# tolerated, so more margin is cheap insurance).
WARM_N = int(_os.environ.get("WARM_N", "9"))
# how many d-tiles into the stream to defer the cent/bv DMA (paced by
# the sync queue's weight-trigger FIFO ahead of it)
CENT_AT = int(_os.environ.get("CENT_AT", "24"))
# number of leading d-tiles processed j-major (2*BLK ps1 banks + 2 ps2
# banks must fit the 8 PSUM banks -> BLK <= 3)
BLK = int(_os.environ.get("BLK", "3"))
NPAIR = (DT + 1) // 2        # 40 d-tile pairs for DoubleRow matmul-2
D_PAD2 = NPAIR * 256         # 10240
C_PAD = 112                  # DoubleRow weight AP needs byte-step %16 == 0
# enc pair index where the second enc tile starts (pairs >= SPLIT_PAIR
# are written by the last 3 d-tiles; MM2 reads them ~8us into its run)
SPLIT_PAIR = NPAIR - 2       # 38

# Stash of the last BassKernelResults (exec_time_ns etc.) for test harnesses.
LAST_RUN = None
_NC_CACHE = None


def _build_nc():
    nc = bacc.Bacc("TRN2", target_bir_lowering=False)
    f32 = mybir.dt.float32
    f32r = mybir.dt.float32r
    bf16 = mybir.dt.bfloat16
    fp8 = mybir.dt.float8e4
    SIGN = mybir.ActivationFunctionType.Sign
    COPY = mybir.ActivationFunctionType.Copy
    DR = mybir.MatmulPerfMode.DoubleRow

    # DRAM I/O (per-core shard layouts, see host prep in kernel()):
    #   fp8_dr path:
    #     sq: [128 ki, KCP, 2, B_SH] fp8   e4m3(samples-0.5).T, k = j*256+jo*128+ki
    #     wt: [DT, 128 ki, KCP, 2, 128 d] fp8  weight.T DR tiles (+/-1), dt >= BLK
    #     wj: [KCP, 128 ki, BLK, 2, 128 d] fp8  j-major slices of tiles 0..BLK-1
    #     bv: [C, 1] f32                   (D - colsum(cent_pm))/2 bias
    #   f32r path:
    #     sf: [128 k_in, KC, B_SH] f32     (samples-0.5).T
    #     wt: [DT, 128 k_in, KC, 128 d_in] f32r  weight.T tiles (+/-1)
    #   ct:  [128 d_in, NPAIR, 2, C_PAD] fp8  centroids.T DR tiles (+/-1)
    #   out: [C, B_SH] f32                 sim.T shard
    fp16 = mybir.dt.float16
    if USE_FP8DR:
        sq_d = nc.dram_tensor("sq", [128, KCP, 2, B_SH], fp8, kind="ExternalInput")
        wt_d = nc.dram_tensor("wt", [DT, 128, KCP, 2, 128], fp8, kind="ExternalInput")
        wj_d = nc.dram_tensor("wj", [KCP, 128, BLK, 2, 128], fp8, kind="ExternalInput")
        bv_d = nc.dram_tensor("bv", [C, 1], f32, kind="ExternalInput")
        lp = fp8
    else:
        lp = {"f32r": f32r, "bf16_hilo": bf16, "fp16": fp16, "bf16": bf16}[MM1_MODE]
        if USE_F32R:
            sf_d = nc.dram_tensor("sf", [128, KC, B_SH], f32, kind="ExternalInput")
            wt_d = nc.dram_tensor("wt", [DT, 128, KC, 128], f32r, kind="ExternalInput")
        elif MM1_MODE == "bf16_hilo":
            sh_d = nc.dram_tensor("sh", [128, KC, B_SH], bf16, kind="ExternalInput")
            sl_d = nc.dram_tensor("sl", [128, KC, B_SH], bf16, kind="ExternalInput")
            wt_d = nc.dram_tensor("wt", [DT, 128, KC, 128], bf16, kind="ExternalInput")
        else:
            sh_d = nc.dram_tensor("sh", [128, KC, B_SH], lp, kind="ExternalInput")
            wt_d = nc.dram_tensor("wt", [DT, 128, KC, 128], lp, kind="ExternalInput")
    ct_d = nc.dram_tensor("ct", [128, NPAIR, 2, C_PAD], fp8, kind="ExternalInput")
    out_d = nc.dram_tensor("out", [C, B_SH], f32, kind="ExternalOutput")

    w_dt = lp

    with tile.TileContext(nc) as tc:
        with (
            tc.tile_pool(name="const", bufs=1) as const_pool,
            tc.tile_pool(name="wts", bufs=8) as w_pool,
            tc.tile_pool(name="outp", bufs=1) as out_pool,
            tc.tile_pool(name="ps1", bufs=3, space=bass.MemorySpace.PSUM) as ps1_pool,
            tc.tile_pool(name="ps2", bufs=1, space=bass.MemorySpace.PSUM) as ps2_pool,
        ):
            if USE_FP8DR and WARM_N > 0:
                # N=512 so the dummy stream is GAPLESS: the HAM activity
                # window only latches warm (K=8/8) under back-to-back
                # saturated streaming; N=128 dummies (~84% busy) never do
                warm = const_pool.tile([128, 2, 512], fp8)
                # memset via uint32 bitcast: 4x fewer stores than byte-wise.
                # On DVE: gpsimd must be free to fire its software-DGE s[1]
                # trigger immediately after the runtime barrier, and DVE
                # needs no activation-table load (ScalarE's COPY does,
                # costing an extra 1.3us before the first dummy).
                # (Skipping the memset entirely doesn't build: the tile
                # framework asserts on read-before-write at release time.)
                nc.vector.memset(warm[:].bitcast(mybir.dt.uint32), 0)
                warm_ps = ps1_pool.tile(
                    [128, 512], mybir.dt.float32, tag="ps1_0", name="warm_ps"
                )
                for _ in range(WARM_N):
                    nc.tensor.matmul(
                        warm_ps[:],
                        warm[:, :, :128],
                        warm[:],
                        start=True,
                        stop=True,
                        perf_mode=DR,
                    )
            if USE_FP8DR:
                s_q = const_pool.tile([128, KCP, 2, B_SH], fp8)
                wjt = const_pool.tile([128, KCP, BLK, 2, 128], fp8)
                # 9 early transfers in consumption order, hand-assigned to
                # the two HWDGE trigger queues. Only s[j0] is split into
                # halves (it alone gates the first real matmul); s[j1..3]
                # go as single 256KB transfers — more, smaller transfers
                # would hit the framework's DMA-semaphore-reuse stalls
                # (trigger #5 on a queue can only issue once transfer #1
                # completes), which is what starves the stream and — far
                # worse — resets the HAM clock gate back to 1.2 GHz.
                early = [
                    # (queue, dst, src), in consumption order. The first
                    # transfer on each HWDGE queue takes ~3.5-4us for
                    # 128KB regardless of size (queue spin-up), so wj0 /
                    # s00 — the first-real-matmul gates — must be FIRST
                    # on their queues. s[1] rides the gpsimd software DGE
                    # as a third parallel channel through the ramp window
                    # (the two HWDGE queues each only deliver ~70GB/s
                    # before ~13us; s[1] on either of them lands ~1.7us
                    # after the j1 pass wants it).
                    (nc.scalar, wjt[:, 0], wj_d[0]),
                    (nc.sync, s_q[:, 0, :, bass.ts(0, 512)],
                     sq_d[:, 0, :, bass.ts(0, 512)]),
                    (nc.gpsimd, s_q[:, 1], sq_d[:, 1]),
                    (nc.scalar, s_q[:, 0, :, bass.ts(1, 512)],
                     sq_d[:, 0, :, bass.ts(1, 512)]),
                    (nc.sync, wjt[:, 1], wj_d[1]),
                    (nc.scalar, wjt[:, 2], wj_d[2]),
                    (nc.sync, s_q[:, 2], sq_d[:, 2]),
                    (nc.scalar, wjt[:, 3], wj_d[3]),
                    (nc.sync, s_q[:, 3], sq_d[:, 3]),
                ]
                for eng, dst, src in early:
                    eng.dma_start(dst, src)
                bv_t = const_pool.tile([C, 1], f32)
            elif USE_F32R:
                s_f = const_pool.tile([128, KC, B_SH], f32)
                s_r = const_pool.tile([128, KC, B_SH], f32r)
                # per-kc loads + f32->f32r rounding casts (DVE is otherwise
                # idle); split so PE can start after the first chunk.
                for b in range(NB):
                    nc.sync.dma_start(
                        s_f[:, 0, bass.ts(b, 512)], sf_d[:, 0, bass.ts(b, 512)]
                    )
                    nc.vector.tensor_copy(
                        s_r[:, 0, bass.ts(b, 512)], s_f[:, 0, bass.ts(b, 512)]
                    )

                def preamble_rest():
                    for kc in range(1, KC):
                        nc.sync.dma_start(s_f[:, kc, :], sf_d[:, kc, :])
                        nc.vector.tensor_copy(s_r[:, kc, :], s_f[:, kc, :])

                w00 = const_pool.tile([128, 128], f32r)
                nc.sync.dma_start(w00[:], wt_d[0, :, 0, :])
                s_streams = [s_r]
            elif MM1_MODE == "bf16_hilo":
                s_hi = const_pool.tile([128, KC, B_SH], bf16)
                s_lo = const_pool.tile([128, KC, B_SH], bf16)
                for kc in range(KC):
                    nc.sync.dma_start(s_hi[:, kc, :], sh_d[:, kc, :])
                    nc.sync.dma_start(s_lo[:, kc, :], sl_d[:, kc, :])
                s_streams = [s_hi, s_lo]
            else:
                s_hi = const_pool.tile([128, KC, B_SH], lp)
                for kc in range(KC):
                    nc.sync.dma_start(s_hi[:, kc, :], sh_d[:, kc, :])
                s_streams = [s_hi]

            cent = const_pool.tile([128, NPAIR, 2, C_PAD], fp8)
            # all sign-encodings buffered on-chip; matmul-2 runs as one
            # uniform fp8-DoubleRow block after the matmul-1 stream ends.
            # Two tiles split at SPLIT_PAIR: MM2's first instructions only
            # dep on enc_a, whose last write lands ~2.5us before the MM1
            # stream ends, so MM2 starts without waiting on the final
            # d-tile's sign op.
            enc_a = const_pool.tile([128, SPLIT_PAIR, 2, B_SH], fp8)
            enc_b = const_pool.tile([128, NPAIR - SPLIT_PAIR, 2, B_SH], fp8)

            def enc_slice(pr, jj, cols):
                if pr < SPLIT_PAIR:
                    return enc_a[:, pr, jj, cols]
                return enc_b[:, pr - SPLIT_PAIR, jj, cols]

            # phantom j=1 half of the final d-pair (dt=79 doesn't exist):
            # zero it so 0-weight x garbage(NaN) can't poison the PSUM
            # (on DVE, keeping gpsimd clear for its early s[1] DMA trigger)
            nc.vector.memset(
                enc_b[:, NPAIR - 1 - SPLIT_PAIR, 1, :].bitcast(mybir.dt.uint32), 0
            )

            def encode(dt, b, ps):
                dst = enc_slice(dt // 2, dt % 2, bass.ts(b, 512))
                if USE_FP8DR and SIGN_SPLIT and b == 1:
                    # DVE: enc in {0,1}; corrected via bv in the output
                    nc.vector.tensor_scalar(
                        dst, ps[:], 0.0, None, mybir.AluOpType.is_gt
                    )
                else:
                    nc.scalar.activation(dst, ps[:], SIGN)

            # (PSUM is bank-granular: per-half ps2 tiles would need a 3rd
            # bank that isn't there — the halves share one tile and eat a
            # one-off ~0.6us dep-serialization blip at the h0->h1 switch)
            ps2 = [
                ps2_pool.tile([C_PAD, 512], mybir.dt.float32, tag=f"ps2_{b}", name=f"ps2_{b}")
                for b in range(NB)
            ]

            if USE_FP8DR:
                # ---- leading BLK d-tiles, j-major ----
                blk_ps = [
                    [
                        ps1_pool.tile(
                            [128, 512], mybir.dt.float32,
                            tag=f"ps1_{b}", name=f"ps1b_{t}_{b}",
                        )
                        for b in range(NB)
                    ]
                    for t in range(BLK)
                ]
                for j in range(KCP):
                    # b-outer for the early passes (s[j0] arrives as two
                    # b-halves); the LAST pass runs t-outer so tile t0
                    # finishes (and encodes) first — dt=BLK reuses t0's
                    # ps1 ring slots and would otherwise wait ~0.3us on
                    # t0's encodes after the pass ends.
                    if j < KCP - 1:
                        order = [(b, t) for b in range(NB) for t in range(BLK)]
                    else:
                        order = [(b, t) for t in range(BLK) for b in range(NB)]
                    for b, t in order:
                        nc.tensor.matmul(
                            blk_ps[t][b][:],
                            wjt[:, j, t],
                            s_q[:, j, :, bass.ts(b, 512)],
                            start=(j == 0),
                            stop=(j == KCP - 1),
                            perf_mode=DR,
                        )
                for t in range(BLK):
                    for b in range(NB):
                        encode(t, b, blk_ps[t][b])

            dt0 = BLK if USE_FP8DR else 0
            for dt in range(dt0, DT):
                if USE_FP8DR:
                    w = w_pool.tile([128, KCP, 2, 128], fp8, tag="w", name=f"w_{dt}")
                    # single DMA per tile: the sync engine serializes
                    # trigger instructions at ~650ns each, and transfers
                    # are striped across all 16 queues anyway
                    nc.sync.dma_start(w[:], wt_d[dt])
                else:
                    w = w_pool.tile([128, KC, 128], w_dt, tag="w", name=f"w_{dt}")
                    nc.sync.dma_start(w[:, : KC // 2, :], wt_d[dt, :, : KC // 2, :])
                    nc.sync.dma_start(w[:, KC // 2 :, :], wt_d[dt, :, KC // 2 :, :])
                if dt == dt0 and not USE_FP8DR:
                    if USE_F32R:
                        preamble_rest()
                    nc.sync.dma_start(cent[:], ct_d[:])
                if USE_FP8DR and dt == CENT_AT:
                    # centroids deferred out of the startup DMA window.
                    # MUST be on the sync queue between weight triggers:
                    # the list scheduler keeps it behind the preceding w
                    # triggers' FIFO there, whereas on the scalar queue
                    # (whose only earlier work is the startup triggers)
                    # it hoists into the ramp window and steals ~1.2MB
                    # of early bandwidth.
                    nc.sync.dma_start(cent[:], ct_d[:])
                    nc.sync.dma_start(bv_t[:], bv_d[:])
                ps1 = [
                    ps1_pool.tile(
                        [128, 512], mybir.dt.float32, tag=f"ps1_{b}", name=f"ps1_{dt}_{b}"
                    )
                    for b in range(NB)
                ]
                if USE_FP8DR:
                    for j in range(KCP):
                        for b in range(NB):
                            nc.tensor.matmul(
                                ps1[b][:],
                                w[:, j, :, :],
                                s_q[:, j, :, bass.ts(b, 512)],
                                start=(j == 0),
                                stop=(j == KCP - 1),
                                perf_mode=DR,
                            )
                else:
                    n_acc = len(s_streams) * KC
                    acc = 0
                    for kc in range(KC):
                        w_src = w00 if (USE_F32R and dt == 0 and kc == 0) else w[:, kc, :]
                        for s_t in s_streams:
                            for b in range(NB):
                                nc.tensor.matmul(
                                    ps1[b][:],
                                    w_src,
                                    s_t[:, kc, bass.ts(b, 512)],
                                    start=(acc == 0),
                                    stop=(acc == n_acc - 1),
                                )
                            acc += 1
                for b in range(NB):
                    encode(dt, b, ps1[b])

            # fp8 block order [b0-h0, b1, b0-h1]: ps2[0] is shared by both
            # b0 halves (no third PSUM bank exists), so h1's first matmul
            # has a WAR dep on h0's output-act READ of the tile — the
            # 8.6us b1 block in between hides it completely (running the
            # halves back-to-back costs a ~0.6us PE gap). Every block's
            # output act+DMA overlaps the next block's matmuls; only the
            # final (b0-h1) chain is exposed, so its 102KB output DMA is
            # split across both trigger queues (single act: two engines
            # reading one PSUM tile get serialized by the framework's
            # consumer chain anyway).
            def mm2_block(b, cols, enc_cols):
                for t in range(NPAIR):
                    nc.tensor.matmul(
                        ps2[b][:, cols],
                        cent[:, t, :, :],
                        enc_slice(t, slice(None), enc_cols),
                        start=(t == 0),
                        stop=(t == NPAIR - 1),
                        perf_mode=DR,
                    )

            if USE_FP8DR:
                # --- b0 h0 ---
                mm2_block(0, slice(0, 256), slice(0, 256))
                ob00 = out_pool.tile([C, 256], mybir.dt.float32, name="ob00")
                nc.scalar.activation(
                    ob00[:], ps2[0][:C, 0:256], COPY, bias=D / 2.0, scale=0.5
                )
                nc.scalar.dma_start(out_d[:, 0:256], ob00[:])
                # --- b1 (full 512) ---
                mm2_block(1, slice(0, 512), slice(512, 1024))
                ob1 = out_pool.tile([C, 512], mybir.dt.float32, name="ob1")
                if SIGN_SPLIT:
                    # enc in {0,1}: sim = ps2 + (D - colsum)/2, on DVE so
                    # it runs while ScalarE handles the b0 halves
                    nc.vector.tensor_scalar(
                        ob1[:], ps2[1][:C, :], bv_t[:], None, mybir.AluOpType.add
                    )
                else:
                    nc.scalar.activation(
                        ob1[:], ps2[1][:C, :], COPY, bias=D / 2.0, scale=0.5
                    )
                nc.scalar.dma_start(out_d[:, 512:1024], ob1[:])
                # --- b0 h1 (final; h0's act read completed during b1) ---
                mm2_block(0, slice(256, 512), slice(256, 512))
                ob01 = out_pool.tile([C, 256], mybir.dt.float32, name="ob01")
                nc.scalar.activation(
                    ob01[:], ps2[0][:C, 256:512], COPY, bias=D / 2.0, scale=0.5
                )
                nc.scalar.dma_start(out_d[:, 256:384], ob01[:, 0:128])
                nc.sync.dma_start(out_d[:, 384:512], ob01[:, 128:256])
            else:
                for b in range(NB):
                    mm2_block(b, slice(0, 512), slice(b * 512, (b + 1) * 512))
                    ob = out_pool.tile(
                        [C, 512], mybir.dt.float32, tag=f"ob_{b}", name=f"ob_{b}"
                    )
                    nc.scalar.activation(
                        ob[:], ps2[b][:C, :], COPY, bias=D / 2.0, scale=0.5
                    )
                    trig = nc.scalar if b == 1 else nc.sync
                    trig.dma_start(out_d[:, b * 512 : (b + 1) * 512], ob[:])

    nc.compile()
    return nc


def _get_nc():
    global _NC_CACHE
    if _NC_CACHE is None:
        _NC_CACHE = _build_nc()
    return _NC_CACHE


def kernel(samples, weight, centroids):
    global LAST_RUN
    samples = np.asarray(samples, dtype=np.float32)
    weight = np.asarray(weight, dtype=np.float32)
    centroids = np.asarray(centroids)

    # ---- host-side marshalling (layout + dtype only) ----
    # centered samples, transposed to [IN_F, B]
    scT = (samples - np.float32(CENTER)).T

    # DoubleRow centroid tiles: ct[d_in, t, j, c] = cent_pm[c, t*256+j*128+d_in]
    cent_pm = np.where(centroids, np.float32(1.0), np.float32(-1.0))
    cpad = np.zeros((NPAIR * 256, C_PAD), dtype=np.float32)
    cpad[:D, :C] = cent_pm.T
    ct = np.ascontiguousarray(
        cpad.reshape(NPAIR, 2, 128, C_PAD).transpose(2, 0, 1, 3).astype(FP8)
    )

    if USE_FP8DR:
        sq8 = scT.astype(FP8)

        def s_core(c):
            # [IN_F, B_SH] -> [128 ki, KCP, 2, B_SH], k = j*256 + jo*128 + ki
            blk = sq8[:, c * B_SH : (c + 1) * B_SH]
            return np.ascontiguousarray(
                blk.reshape(KCP, 2, 128, B_SH).transpose(2, 0, 1, 3)
            )

        # weight.T DR tiles: wt[dt, ki, j, jo, d_in] = w[dt*128+d_in, j*256+jo*128+ki]
        wpad = np.zeros((D_PAD, IN_F), dtype=FP8)
        wpad[:D] = weight.astype(FP8)  # +/-1, exact in fp8
        wt = np.ascontiguousarray(
            wpad.reshape(DT, 128, KCP, 2, 128).transpose(0, 4, 2, 3, 1)
        )
        # j-major first-block slices: wj[j, ki, t, jo, d_in] = wt[t, ki, j, jo, d_in]
        wj = np.ascontiguousarray(wt[:BLK].transpose(2, 1, 0, 3, 4))
        # bias vector for the {0,1}-encoded chunk: (D - colsum(cent_pm))/2
        bv = (
            (np.float32(D) - cent_pm.sum(axis=1, dtype=np.float32)) * 0.5
        ).astype(np.float32)[:, None]
        in_maps = [
            {"sq": s_core(c), "wt": wt, "wj": wj, "ct": ct, "bv": bv}
            for c in range(N_CORES)
        ]
    else:
        FP16 = np.float16
        w_np = {"f32r": np.float32, "bf16_hilo": BF16, "fp16": FP16, "bf16": BF16}[
            MM1_MODE
        ]

        def s_core_legacy(a, c):
            # [IN_F, B_SH] -> [128 k_in, KC, B_SH]
            blk = a[:, c * B_SH : (c + 1) * B_SH]
            return np.ascontiguousarray(blk.reshape(KC, 128, B_SH).transpose(1, 0, 2))

        # weight.T tiles: wt[dt, k_in, kc, d_in] = weight[dt*128+d_in, kc*128+k_in]
        wpad = np.zeros((D_PAD, IN_F), dtype=w_np)
        wpad[:D] = weight.astype(w_np)  # +/-1, exact in bf16/f32r
        wt = np.ascontiguousarray(wpad.reshape(DT, 128, KC, 128).transpose(0, 3, 2, 1))

        if USE_F32R:
            in_maps = [
                {"sf": s_core_legacy(scT, c), "wt": wt, "ct": ct}
                for c in range(N_CORES)
            ]
        elif MM1_MODE == "bf16_hilo":
            s_hi = scT.astype(BF16)
            s_lo = (scT - s_hi.astype(np.float32)).astype(BF16)
            in_maps = [
                {
                    "sh": s_core_legacy(s_hi, c),
                    "sl": s_core_legacy(s_lo, c),
                    "wt": wt,
                    "ct": ct,
                }
                for c in range(N_CORES)
            ]
        else:
            s_hi = scT.astype(w_np)
            in_maps = [
                {"sh": s_core_legacy(s_hi, c), "wt": wt, "ct": ct}
                for c in range(N_CORES)
            ]

    nc = _get_nc()
    res = run_bass_kernel_spmd(nc, in_maps, core_ids=list(range(N_CORES)))
    LAST_RUN = res

    # gather: out[c] is sim.T for batch rows [c*B_SH, (c+1)*B_SH)
    return np.vstack(
        [np.asarray(res.results[c]["out"]).T for c in range(N_CORES)]
    ).astype(np.float32)


# revision 15
# speedup vs baseline: 1.0030x; 1.0030x over previous
"""Trainium2 Bass kernel for nn_Classifier_48223892799748 (retrieval_knn).

Computes sim = (D + enc_pm @ cent_pm.T) / 2 where
  enc_pm = sign((samples - 0.5) @ weight.T)  in {+1,-1}
  cent_pm = centroids mapped {0,1} -> {-1,+1}

Sharding: data-parallel over the batch dim (8192 -> 1024 rows per core,
8 cores). weight / centroids replicated.

Device layout: everything is computed transposed ([D, B] / [C, B]) so that
the sign-encoded matmul-1 output tile [128 d, 512 b] feeds matmul-2
directly as the moving operand (contraction over d) with no on-device
transpose.

Matmul-1 runs in fp8e4m3 DoubleRow (2x bf16 MAC rate; the stream floor
is 512 cols @2.4GHz = ~216ns per [256k x 128d x 512b] matmul): samples
are quantized to e4m3 on the host, weights (+/-1) are exact in fp8.
This flips ~85 of the 10000 sign bits per batch row (quantization noise
~0.64 vs proj sigma ~36), moving each output count by at most ~47 of an
allowed ~105 (rel gate 2e-2 at scale 5260): measured rel err 0.0089,
identical on host emulation and hardware. Matmul-2 is exact.

Sign encodings are split across engines so neither gates the PE stream:
  b-chunk 0: ScalarE Sign        -> enc in {+1,-1}; out = 0.5*ps2 + D/2
  b-chunk 1: DVE    is_gt(0)     -> enc in {0,1};   out = ps2 + (D-colsum)/2
(the {0,1} encoding needs the per-centroid colsum correction, shipped as
a tiny [C,1] bias vector computed on host).

Startup choreography (the DMA ramp only delivers ~50->350 GB/s over the
first 7-14us, while a dt-major d-tile burns the full 1MB sample set in
1.7us):
  - the first BLK(=3) d-tiles run J-MAJOR: each j-pass streams one 256KB
    sample chunk against per-j weight slices of all BLK tiles (separate
    `wj` DRAM layout, one 96KB DMA per j), dropping the early demand
    rate ~3x so the real stream can start ~1.5us earlier and run with
    far fewer starvation gaps. Uses all 8 PSUM banks (2*BLK ps1 + 2 ps2).
  - the 12 early transfers (4 wj slices + 8 sample half-chunks) are
    issued in consumption order, strictly alternating between the two
    HWDGE trigger queues (scalar, sync) to balance the ramp.
  - WARM_N dummy N=512 DR matmuls on a memset tile at the head of the PE
    queue: the HAM clock gate only reaches 8/8 (2.4 GHz) after ~3.4us of
    GAPLESS streaming, and the DMA cannot feed real work before ~10us
    (fixed ~7us runtime preamble + ramp).
  - the 1.2MB centroid load is deferred (program-order paced via
    CENT_AT) out of the startup DMA window; MM2 needs it ~150us in.
  - single DMA per 128KB steady-state weight tile (transfers stripe
    across all 16 queues; more triggers only serialize the sync engine).

enc is stored in two tiles split at pair SPLIT_PAIR so matmul-2's first
instructions (which read early pairs) don't dep-serialize on the LAST
d-tile's sign op (the tile dep tracker is coarser than a pair slice).

The MM2 block order is b-major, b=1 first: the earlier chunk's output
path overlaps the later chunk's matmul block. The last block runs as two
256-column halves, and the final half's output is further split into two
128-col quarters processed on ScalarE and DVE in parallel with their
DMAs on different trigger queues, shortening the serial post-matmul tail.

Legacy modes (f32r / bf16_hilo / fp16 / bf16 matmul-1) remain selectable
via MM1_MODE for A/B testing.
"""

import sys

if "/opt/trn_rl_repo" not in sys.path:
    sys.path.insert(0, "/opt/trn_rl_repo")

import ml_dtypes
import numpy as np

import concourse.bass as bass
import concourse.mybir as mybir
import concourse.tile as tile
from concourse import bacc
from concourse.bass_utils import run_bass_kernel_spmd

# The container's `antenv` package is a stub without `axon_hooks`; if tracing
# is ever requested (BASS_TRACE=1), run_bass_kernel_spmd imports it and would
# crash. Provide a stub module (hook=None -> tracing skipped gracefully)
# unless something (e.g. a test harness) registered a real one already.
try:  # pragma: no cover
    import antenv.axon_hooks  # noqa: F401
except ImportError:
    import types as _types

    import antenv as _antenv

    _hooks = _types.ModuleType("antenv.axon_hooks")
    _hook_store = {"h": None}
    _hooks.set_axon_ntff_profile_hook = lambda h: _hook_store.__setitem__("h", h)
    _hooks.get_axon_ntff_profile_hook = lambda: _hook_store["h"]
    sys.modules["antenv.axon_hooks"] = _hooks
    _antenv.axon_hooks = _hooks

BF16 = ml_dtypes.bfloat16
FP8 = ml_dtypes.float8_e4m3

B, IN_F, D, C = 8192, 1024, 10000, 100
N_CORES = 8
B_SH = B // N_CORES          # 1024 batch rows per core
KC = IN_F // 128             # 8 contraction chunks for matmul 1
KCP = KC // 2                # 4 DoubleRow contraction pairs
DT = (D + 127) // 128        # 79 d-tiles
D_PAD = DT * 128             # 10112
NB = B_SH // 512             # 2 psum-width chunks of the local batch
CENTER = 0.5

# matmul-1 mode: "fp8_dr" | "f32r" | "bf16_hilo" | "fp16" | "bf16"
import os as _os
MM1_MODE = _os.environ.get("MM1_MODE", "fp8_dr")
USE_F32R = MM1_MODE == "f32r"
USE_FP8DR = MM1_MODE == "fp8_dr"
# split sign-encoding across ScalarE+DVE (fp8_dr path only)
SIGN_SPLIT = _os.environ.get("SIGN_SPLIT", "1") == "1"
# dummy warm-up matmuls at the head of the PE queue (see module docstring).
# 9 covers the observed 11.0-11.8us arrival spread of the first sample
# chunk: if the dummies run out first, the pre-latch gap resets the HAM
# clock ramp and costs ~2.5us of 1.2GHz matmuls (post-latch gaps are
# tolerated, so more margin is cheap insurance).
WARM_N = int(_os.environ.get("WARM_N", "9"))
# how many d-tiles into the stream to defer the cent/bv DMA (paced by
# the sync queue's weight-trigger FIFO ahead of it)
CENT_AT = int(_os.environ.get("CENT_AT", "24"))
# number of leading d-tiles processed j-major (2*BLK ps1 banks + 2 ps2
# banks must fit the 8 PSUM banks -> BLK <= 3)
BLK = int(_os.environ.get("BLK", "3"))
NPAIR = (DT + 1) // 2        # 40 d-tile pairs for DoubleRow matmul-2
D_PAD2 = NPAIR * 256         # 10240
C_PAD = 112                  # DoubleRow weight AP needs byte-step %16 == 0
# enc pair index where the second enc tile starts (pairs >= SPLIT_PAIR
# are written by the last 3 d-tiles; MM2 reads them ~8us into its run)
SPLIT_PAIR = NPAIR - 2       # 38

# Stash of the last BassKernelResults (exec_time_ns etc.) for test harnesses.
LAST_RUN = None
_NC_CACHE = None


def _build_nc():
    nc = bacc.Bacc("TRN2", target_bir_lowering=False)
    f32 = mybir.dt.float32
    f32r = mybir.dt.float32r
    bf16 = mybir.dt.bfloat16
    fp8 = mybir.dt.float8e4
    SIGN = mybir.ActivationFunctionType.Sign
    COPY = mybir.ActivationFunctionType.Copy
    DR = mybir.MatmulPerfMode.DoubleRow

    # DRAM I/O (per-core shard layouts, see host prep in kernel()):
    #   fp8_dr path:
    #     sq: [128 ki, KCP, 2, B_SH] fp8   e4m3(samples-0.5).T, k = j*256+jo*128+ki
    #     wt: [DT, 128 ki, KCP, 2, 128 d] fp8  weight.T DR tiles (+/-1), dt >= BLK
    #     wj: [KCP, 128 ki, BLK, 2, 128 d] fp8  j-major slices of tiles 0..BLK-1
    #     bv: [C, 1] f32                   (D - colsum(cent_pm))/2 bias
    #   f32r path:
    #     sf: [128 k_in, KC, B_SH] f32     (samples-0.5).T
    #     wt: [DT, 128 k_in, KC, 128 d_in] f32r  weight.T tiles (+/-1)
    #   ct:  [128 d_in, NPAIR, 2, C_PAD] fp8  centroids.T DR tiles (+/-1)
    #   out: [C, B_SH] f32                 sim.T shard
    fp16 = mybir.dt.float16
    if USE_FP8DR:
        sq_d = nc.dram_tensor("sq", [128, KCP, 2, B_SH], fp8, kind="ExternalInput")
        wt_d = nc.dram_tensor("wt", [DT, 128, KCP, 2, 128], fp8, kind="ExternalInput")
        wj_d = nc.dram_tensor("wj", [KCP, 128, BLK, 2, 128], fp8, kind="ExternalInput")
        bv_d = nc.dram_tensor("bv", [C, 1], f32, kind="ExternalInput")
        lp = fp8
    else:
        lp = {"f32r": f32r, "bf16_hilo": bf16, "fp16": fp16, "bf16": bf16}[MM1_MODE]
        if USE_F32R:
            sf_d = nc.dram_tensor("sf", [128, KC, B_SH], f32, kind="ExternalInput")
            wt_d = nc.dram_tensor("wt", [DT, 128, KC, 128], f32r, kind="ExternalInput")
        elif MM1_MODE == "bf16_hilo":
            sh_d = nc.dram_tensor("sh", [128, KC, B_SH], bf16, kind="ExternalInput")
            sl_d = nc.dram_tensor("sl", [128, KC, B_SH], bf16, kind="ExternalInput")
            wt_d = nc.dram_tensor("wt", [DT, 128, KC, 128], bf16, kind="ExternalInput")
        else:
            sh_d = nc.dram_tensor("sh", [128, KC, B_SH], lp, kind="ExternalInput")
            wt_d = nc.dram_tensor("wt", [DT, 128, KC, 128], lp, kind="ExternalInput")
    ct_d = nc.dram_tensor("ct", [128, NPAIR, 2, C_PAD], fp8, kind="ExternalInput")
    out_d = nc.dram_tensor("out", [C, B_SH], f32, kind="ExternalOutput")

    w_dt = lp

    with tile.TileContext(nc) as tc:
        with (
            tc.tile_pool(name="const", bufs=1) as const_pool,
            tc.tile_pool(name="wts", bufs=8) as w_pool,
            tc.tile_pool(name="outp", bufs=1) as out_pool,
            tc.tile_pool(name="ps1", bufs=3, space=bass.MemorySpace.PSUM) as ps1_pool,
            tc.tile_pool(name="ps2", bufs=1, space=bass.MemorySpace.PSUM) as ps2_pool,
        ):
            if USE_FP8DR and WARM_N > 0:
                # N=512 so the dummy stream is GAPLESS: the HAM activity
                # window only latches warm (K=8/8) under back-to-back
                # saturated streaming; N=128 dummies (~84% busy) never do
                warm = const_pool.tile([128, 2, 512], fp8)
                # memset via uint32 bitcast: 4x fewer stores than byte-wise.
                # On gpsimd: it reaches its first user instruction ~190ns
                # before DVE does, and the warm-up stream's start time sets
                # when the clock latches; gpsimd's s[1] DMA trigger follows
                # right after (the phantom memset that used to sit between
                # them lives on DVE now).
                # (Skipping the memset entirely doesn't build: the tile
                # framework asserts on read-before-write at release time.)
                nc.gpsimd.memset(warm[:].bitcast(mybir.dt.uint32), 0)
                warm_ps = ps1_pool.tile(
                    [128, 512], mybir.dt.float32, tag="ps1_0", name="warm_ps"
                )
                for _ in range(WARM_N):
                    nc.tensor.matmul(
                        warm_ps[:],
                        warm[:, :, :128],
                        warm[:],
                        start=True,
                        stop=True,
                        perf_mode=DR,
                    )
            if USE_FP8DR:
                s_q = const_pool.tile([128, KCP, 2, B_SH], fp8)
                wjt = const_pool.tile([128, KCP, BLK, 2, 128], fp8)
                # 9 early transfers in consumption order, hand-assigned to
                # the two HWDGE trigger queues. Only s[j0] is split into
                # halves (it alone gates the first real matmul); s[j1..3]
                # go as single 256KB transfers — more, smaller transfers
                # would hit the framework's DMA-semaphore-reuse stalls
                # (trigger #5 on a queue can only issue once transfer #1
                # completes), which is what starves the stream and — far
                # worse — resets the HAM clock gate back to 1.2 GHz.
                early = [
                    # (queue, dst, src), in consumption order. The first
                    # transfer on each HWDGE queue takes ~3.5-4us for
                    # 128KB regardless of size (queue spin-up), so wj0 /
                    # s00 — the first-real-matmul gates — must be FIRST
                    # on their queues. s[1] rides the gpsimd software DGE
                    # as a third parallel channel through the ramp window
                    # (the two HWDGE queues each only deliver ~70GB/s
                    # before ~13us; s[1] on either of them lands ~1.7us
                    # after the j1 pass wants it).
                    (nc.scalar, wjt[:, 0], wj_d[0]),
                    (nc.sync, s_q[:, 0, :, bass.ts(0, 512)],
                     sq_d[:, 0, :, bass.ts(0, 512)]),
                    (nc.gpsimd, s_q[:, 1], sq_d[:, 1]),
                    (nc.scalar, s_q[:, 0, :, bass.ts(1, 512)],
                     sq_d[:, 0, :, bass.ts(1, 512)]),
                    (nc.sync, wjt[:, 1], wj_d[1]),
                    (nc.scalar, wjt[:, 2], wj_d[2]),
                    (nc.sync, s_q[:, 2], sq_d[:, 2]),
                    (nc.scalar, wjt[:, 3], wj_d[3]),
                    (nc.sync, s_q[:, 3], sq_d[:, 3]),
                ]
                for eng, dst, src in early:
                    eng.dma_start(dst, src)
                bv_t = const_pool.tile([C, 1], f32)
            elif USE_F32R:
                s_f = const_pool.tile([128, KC, B_SH], f32)
                s_r = const_pool.tile([128, KC, B_SH], f32r)
                # per-kc loads + f32->f32r rounding casts (DVE is otherwise
                # idle); split so PE can start after the first chunk.
                for b in range(NB):
                    nc.sync.dma_start(
                        s_f[:, 0, bass.ts(b, 512)], sf_d[:, 0, bass.ts(b, 512)]
                    )
                    nc.vector.tensor_copy(
                        s_r[:, 0, bass.ts(b, 512)], s_f[:, 0, bass.ts(b, 512)]
                    )

                def preamble_rest():
                    for kc in range(1, KC):
                        nc.sync.dma_start(s_f[:, kc, :], sf_d[:, kc, :])
                        nc.vector.tensor_copy(s_r[:, kc, :], s_f[:, kc, :])

                w00 = const_pool.tile([128, 128], f32r)
                nc.sync.dma_start(w00[:], wt_d[0, :, 0, :])
                s_streams = [s_r]
            elif MM1_MODE == "bf16_hilo":
                s_hi = const_pool.tile([128, KC, B_SH], bf16)
                s_lo = const_pool.tile([128, KC, B_SH], bf16)
                for kc in range(KC):
                    nc.sync.dma_start(s_hi[:, kc, :], sh_d[:, kc, :])
                    nc.sync.dma_start(s_lo[:, kc, :], sl_d[:, kc, :])
                s_streams = [s_hi, s_lo]
            else:
                s_hi = const_pool.tile([128, KC, B_SH], lp)
                for kc in range(KC):
                    nc.sync.dma_start(s_hi[:, kc, :], sh_d[:, kc, :])
                s_streams = [s_hi]

            cent = const_pool.tile([128, NPAIR, 2, C_PAD], fp8)
            # all sign-encodings buffered on-chip; matmul-2 runs as one
            # uniform fp8-DoubleRow block after the matmul-1 stream ends.
            # Two tiles split at SPLIT_PAIR: MM2's first instructions only
            # dep on enc_a, whose last write lands ~2.5us before the MM1
            # stream ends, so MM2 starts without waiting on the final
            # d-tile's sign op.
            enc_a = const_pool.tile([128, SPLIT_PAIR, 2, B_SH], fp8)
            enc_b = const_pool.tile([128, NPAIR - SPLIT_PAIR, 2, B_SH], fp8)

            def enc_slice(pr, jj, cols):
                if pr < SPLIT_PAIR:
                    return enc_a[:, pr, jj, cols]
                return enc_b[:, pr - SPLIT_PAIR, jj, cols]

            # phantom j=1 half of the final d-pair (dt=79 doesn't exist):
            # zero it so 0-weight x garbage(NaN) can't poison the PSUM
            # (on DVE, keeping gpsimd clear for its early s[1] DMA trigger)
            nc.vector.memset(
                enc_b[:, NPAIR - 1 - SPLIT_PAIR, 1, :].bitcast(mybir.dt.uint32), 0
            )

            def encode(dt, b, ps):
                dst = enc_slice(dt // 2, dt % 2, bass.ts(b, 512))
                if USE_FP8DR and SIGN_SPLIT and b == 1:
                    # DVE: enc in {0,1}; corrected via bv in the output
                    nc.vector.tensor_scalar(
                        dst, ps[:], 0.0, None, mybir.AluOpType.is_gt
                    )
                else:
                    nc.scalar.activation(dst, ps[:], SIGN)

            # (PSUM is bank-granular: per-half ps2 tiles would need a 3rd
            # bank that isn't there — the halves share one tile and eat a
            # one-off ~0.6us dep-serialization blip at the h0->h1 switch)
            ps2 = [
                ps2_pool.tile([C_PAD, 512], mybir.dt.float32, tag=f"ps2_{b}", name=f"ps2_{b}")
                for b in range(NB)
            ]

            if USE_FP8DR:
                # ---- leading BLK d-tiles, j-major ----
                blk_ps = [
                    [
                        ps1_pool.tile(
                            [128, 512], mybir.dt.float32,
                            tag=f"ps1_{b}", name=f"ps1b_{t}_{b}",
                        )
                        for b in range(NB)
                    ]
                    for t in range(BLK)
                ]
                for j in range(KCP):
                    # b-outer for the early passes (s[j0] arrives as two
                    # b-halves); the LAST pass runs t-outer so tile t0
                    # finishes (and encodes) first — dt=BLK reuses t0's
                    # ps1 ring slots and would otherwise wait ~0.3us on
                    # t0's encodes after the pass ends.
                    if j < KCP - 1:
                        order = [(b, t) for b in range(NB) for t in range(BLK)]
                    else:
                        order = [(b, t) for t in range(BLK) for b in range(NB)]
                    for b, t in order:
                        nc.tensor.matmul(
                            blk_ps[t][b][:],
                            wjt[:, j, t],
                            s_q[:, j, :, bass.ts(b, 512)],
                            start=(j == 0),
                            stop=(j == KCP - 1),
                            perf_mode=DR,
                        )
                for t in range(BLK):
                    for b in range(NB):
                        encode(t, b, blk_ps[t][b])

            dt0 = BLK if USE_FP8DR else 0
            for dt in range(dt0, DT):
                if USE_FP8DR:
                    w = w_pool.tile([128, KCP, 2, 128], fp8, tag="w", name=f"w_{dt}")
                    # single DMA per tile: the sync engine serializes
                    # trigger instructions at ~650ns each, and transfers
                    # are striped across all 16 queues anyway
                    nc.sync.dma_start(w[:], wt_d[dt])
                else:
                    w = w_pool.tile([128, KC, 128], w_dt, tag="w", name=f"w_{dt}")
                    nc.sync.dma_start(w[:, : KC // 2, :], wt_d[dt, :, : KC // 2, :])
                    nc.sync.dma_start(w[:, KC // 2 :, :], wt_d[dt, :, KC // 2 :, :])
                if dt == dt0 and not USE_FP8DR:
                    if USE_F32R:
                        preamble_rest()
                    nc.sync.dma_start(cent[:], ct_d[:])
                if USE_FP8DR and dt == CENT_AT:
                    # centroids deferred out of the startup DMA window.
                    # MUST be on the sync queue between weight triggers:
                    # the list scheduler keeps it behind the preceding w
                    # triggers' FIFO there, whereas on the scalar queue
                    # (whose only earlier work is the startup triggers)
                    # it hoists into the ramp window and steals ~1.2MB
                    # of early bandwidth.
                    nc.sync.dma_start(cent[:], ct_d[:])
                    nc.sync.dma_start(bv_t[:], bv_d[:])
                ps1 = [
                    ps1_pool.tile(
                        [128, 512], mybir.dt.float32, tag=f"ps1_{b}", name=f"ps1_{dt}_{b}"
                    )
                    for b in range(NB)
                ]
                if USE_FP8DR:
                    for j in range(KCP):
                        for b in range(NB):
                            nc.tensor.matmul(
                                ps1[b][:],
                                w[:, j, :, :],
                                s_q[:, j, :, bass.ts(b, 512)],
                                start=(j == 0),
                                stop=(j == KCP - 1),
                                perf_mode=DR,
                            )
                else:
                    n_acc = len(s_streams) * KC
                    acc = 0
                    for kc in range(KC):
                        w_src = w00 if (USE_F32R and dt == 0 and kc == 0) else w[:, kc, :]
                        for s_t in s_streams:
                            for b in range(NB):
                                nc.tensor.matmul(
                                    ps1[b][:],
                                    w_src,
                                    s_t[:, kc, bass.ts(b, 512)],
                                    start=(acc == 0),
                                    stop=(acc == n_acc - 1),
                                )
                            acc += 1
                for b in range(NB):
                    encode(dt, b, ps1[b])

            # fp8 block order [b0-h0, b1, b0-h1]: ps2[0] is shared by both
            # b0 halves (no third PSUM bank exists), so h1's first matmul
            # has a WAR dep on h0's output-act READ of the tile — the
            # 8.6us b1 block in between hides it completely (running the
            # halves back-to-back costs a ~0.6us PE gap). Every block's
            # output act+DMA overlaps the next block's matmuls; only the
            # final (b0-h1) chain is exposed, so its 102KB output DMA is
            # split across both trigger queues (single act: two engines
            # reading one PSUM tile get serialized by the framework's
            # consumer chain anyway).
            def mm2_block(b, cols, enc_cols):
                for t in range(NPAIR):
                    nc.tensor.matmul(
                        ps2[b][:, cols],
                        cent[:, t, :, :],
                        enc_slice(t, slice(None), enc_cols),
                        start=(t == 0),
                        stop=(t == NPAIR - 1),
                        perf_mode=DR,
                    )

            if USE_FP8DR:
                # --- b0 h0 ---
                mm2_block(0, slice(0, 256), slice(0, 256))
                ob00 = out_pool.tile([C, 256], mybir.dt.float32, name="ob00")
                nc.scalar.activation(
                    ob00[:], ps2[0][:C, 0:256], COPY, bias=D / 2.0, scale=0.5
                )
                nc.scalar.dma_start(out_d[:, 0:256], ob00[:])
                # --- b1 (full 512) ---
                mm2_block(1, slice(0, 512), slice(512, 1024))
                ob1 = out_pool.tile([C, 512], mybir.dt.float32, name="ob1")
                if SIGN_SPLIT:
                    # enc in {0,1}: sim = ps2 + (D - colsum)/2, on DVE so
                    # it runs while ScalarE handles the b0 halves
                    nc.vector.tensor_scalar(
                        ob1[:], ps2[1][:C, :], bv_t[:], None, mybir.AluOpType.add
                    )
                else:
                    nc.scalar.activation(
                        ob1[:], ps2[1][:C, :], COPY, bias=D / 2.0, scale=0.5
                    )
                nc.scalar.dma_start(out_d[:, 512:1024], ob1[:])
                # --- b0 h1 (final; h0's act read completed during b1) ---
                mm2_block(0, slice(256, 512), slice(256, 512))
                ob01 = out_pool.tile([C, 256], mybir.dt.float32, name="ob01")
                nc.scalar.activation(
                    ob01[:], ps2[0][:C, 256:512], COPY, bias=D / 2.0, scale=0.5
                )
                nc.scalar.dma_start(out_d[:, 256:384], ob01[:, 0:128])
                nc.sync.dma_start(out_d[:, 384:512], ob01[:, 128:256])
            else:
                for b in range(NB):
                    mm2_block(b, slice(0, 512), slice(b * 512, (b + 1) * 512))
                    ob = out_pool.tile(
                        [C, 512], mybir.dt.float32, tag=f"ob_{b}", name=f"ob_{b}"
                    )
                    nc.scalar.activation(
                        ob[:], ps2[b][:C, :], COPY, bias=D / 2.0, scale=0.5
                    )
                    trig = nc.scalar if b == 1 else nc.sync
                    trig.dma_start(out_d[:, b * 512 : (b + 1) * 512], ob[:])

    nc.compile()
    return nc


def _get_nc():
    global _NC_CACHE
    if _NC_CACHE is None:
        _NC_CACHE = _build_nc()
    return _NC_CACHE


def kernel(samples, weight, centroids):
    global LAST_RUN
    samples = np.asarray(samples, dtype=np.float32)
    weight = np.asarray(weight, dtype=np.float32)
    centroids = np.asarray(centroids)

    # ---- host-side marshalling (layout + dtype only) ----
    # centered samples, transposed to [IN_F, B]
    scT = (samples - np.float32(CENTER)).T

    # DoubleRow centroid tiles: ct[d_in, t, j, c] = cent_pm[c, t*256+j*128+d_in]
    cent_pm = np.where(centroids, np.float32(1.0), np.float32(-1.0))
    cpad = np.zeros((NPAIR * 256, C_PAD), dtype=np.float32)
    cpad[:D, :C] = cent_pm.T
    ct = np.ascontiguousarray(
        cpad.reshape(NPAIR, 2, 128, C_PAD).transpose(2, 0, 1, 3).astype(FP8)
    )

    if USE_FP8DR:
        sq8 = scT.astype(FP8)

        def s_core(c):
            # [IN_F, B_SH] -> [128 ki, KCP, 2, B_SH], k = j*256 + jo*128 + ki
            blk = sq8[:, c * B_SH : (c + 1) * B_SH]
            return np.ascontiguousarray(
                blk.reshape(KCP, 2, 128, B_SH).transpose(2, 0, 1, 3)
            )

        # weight.T DR tiles: wt[dt, ki, j, jo, d_in] = w[dt*128+d_in, j*256+jo*128+ki]
        wpad = np.zeros((D_PAD, IN_F), dtype=FP8)
        wpad[:D] = weight.astype(FP8)  # +/-1, exact in fp8
        wt = np.ascontiguousarray(
            wpad.reshape(DT, 128, KCP, 2, 128).transpose(0, 4, 2, 3, 1)
        )
        # j-major first-block slices: wj[j, ki, t, jo, d_in] = wt[t, ki, j, jo, d_in]
        wj = np.ascontiguousarray(wt[:BLK].transpose(2, 1, 0, 3, 4))
        # bias vector for the {0,1}-encoded chunk: (D - colsum(cent_pm))/2
        bv = (
            (np.float32(D) - cent_pm.sum(axis=1, dtype=np.float32)) * 0.5
        ).astype(np.float32)[:, None]
        in_maps = [
            {"sq": s_core(c), "wt": wt, "wj": wj, "ct": ct, "bv": bv}
            for c in range(N_CORES)
        ]
    else:
        FP16 = np.float16
        w_np = {"f32r": np.float32, "bf16_hilo": BF16, "fp16": FP16, "bf16": BF16}[
            MM1_MODE
        ]

        def s_core_legacy(a, c):
            # [IN_F, B_SH] -> [128 k_in, KC, B_SH]
            blk = a[:, c * B_SH : (c + 1) * B_SH]
            return np.ascontiguousarray(blk.reshape(KC, 128, B_SH).transpose(1, 0, 2))

        # weight.T tiles: wt[dt, k_in, kc, d_in] = weight[dt*128+d_in, kc*128+k_in]
        wpad = np.zeros((D_PAD, IN_F), dtype=w_np)
        wpad[:D] = weight.astype(w_np)  # +/-1, exact in bf16/f32r
        wt = np.ascontiguousarray(wpad.reshape(DT, 128, KC, 128).transpose(0, 3, 2, 1))

        if USE_F32R:
            in_maps = [
                {"sf": s_core_legacy(scT, c), "wt": wt, "ct": ct}
                for c in range(N_CORES)
            ]
        elif MM1_MODE == "bf16_hilo":
            s_hi = scT.astype(BF16)
            s_lo = (scT - s_hi.astype(np.float32)).astype(BF16)
            in_maps = [
                {
                    "sh": s_core_legacy(s_hi, c),
                    "sl": s_core_legacy(s_lo, c),
                    "wt": wt,
                    "ct": ct,
                }
                for c in range(N_CORES)
            ]
        else:
            s_hi = scT.astype(w_np)
            in_maps = [
                {"sh": s_core_legacy(s_hi, c), "wt": wt, "ct": ct}
                for c in range(N_CORES)
            ]

    nc = _get_nc()
    res = run_bass_kernel_spmd(nc, in_maps, core_ids=list(range(N_CORES)))
    LAST_RUN = res

    # gather: out[c] is sim.T for batch rows [c*B_SH, (c+1)*B_SH)
    return np.vstack(
        [np.asarray(res.results[c]["out"]).T for c in range(N_CORES)]
    ).astype(np.float32)
